# revision 7
# baseline (speedup 1.0000x reference)
"""CTC batch loss on 8 TRN2 NeuronCores — pure data parallel, log-space DP.

Strategy (v5):
- Batch dim sharded 128 samples/core = SBUF partitions. The 511 sequential
  DP steps are split into a forward alpha chain (t=0..255) and a backward
  beta chain (t=511..255) that MEET at t*=255; both chains live side by
  side in ONE 264-wide state row (fwd state at cols 2..130, bwd state
  REVERSED at cols 133..261), so every step is instructions over a single
  261-wide window covering both chains at once.
- Each LSE2 is one fused custom DVE op (quadratic-softplus approx):
      LSE_QSP(x, y) = max(x,y) + sq(relu(c0 + c1*(max-min)))
  (e2e rel err 2e-3 vs the 2e-2 gate). A second fused op folds the
  backward label-end injection AND the emission add into one instruction:
      INJLP(l2, lp; cinj) = max(l2, window0(Idx - cinj)) + lp
  where window0 yields 0.0 exactly on the 2-element inject window
  [cinj, cinj+1] and -3e38 elsewhere; cinj is a per-partition scalar
  streamed from a tiny [128, 256] table (9999 = no inject).
- Net: 4 DVE instructions per step, no ScalarE/act in the hot loop, no
  cross-engine syncs. Emission log-probs lp are host-gathered into the
  combined layout and shipped bf16 (17 MB/core).
- Also monkeypatches around two toolchain bugs (see comments below):
  instructions with >1 sem waits and the Tile tail drain.
"""
import sys

for _p in ("/opt/trn_rl_repo", "/opt/pypackages"):
    if _p not in sys.path:
        sys.path.insert(0, _p)

import numpy as np
import ml_dtypes

import concourse.bass as bass
import concourse.tile as tile
from concourse import mybir
from concourse.bass_utils import run_bass_kernel_spmd

B, T, C, L = 1024, 512, 128, 64
S = 2 * L + 1          # 129 extended states
NCORES = 8
BL = B // NCORES       # 128 samples per core = SBUF partitions
EPS = 1e-7
NEG = -30000.0

TW = 264               # combined state row width
FO = 2                 # fwd state s at col FO+s        (cols 2..130)
BO = 133               # bwd state s at col 261-s       (cols 133..261)
W = 261                # hot instruction window: cols [2, 263)
NSTEP = 256
# graduated lp chunk sizes (steps): small first chunks so step 0 starts
# ~2us in; all DMAs are issued upfront and arrive ahead of consumption.
CHUNKS = [4, 12, 16, 32, 32, 32, 32, 32, 32, 32]
assert sum(CHUNKS) == NSTEP
CINJ_OFF = 9999.0      # "no injection this step"

F32 = mybir.dt.float32
BF16 = mybir.dt.bfloat16
ALU = mybir.AluOpType
ACTF = mybir.ActivationFunctionType

SP_C0 = 0.8129
SP_C1 = -0.2261
INJ_BIG = -3.0e38

# --- workaround: this walrus build rejects instructions with >2 sem waits
# ("Too many sync wait commands" in CoreV3 codegen). Tile's kernel-tail
# drain aggregates every outstanding token onto one SP Drain; split it
# into a chain of drains each carrying at most MAX_WAITS conditions.
_MAX_WAITS = 1


def _patched_drain_and_barrier(self, tick_clock, wait_clock):
    from concourse.vector_clock import ScopedClock

    drain_inst = self.nc.sync.drain()
    wait_clock.add_sem_waits(
        drain_inst.ins, ScopedClock({None: tick_clock.global_clock})
    )
    si = drain_inst.ins.sync_info
    waits = list(si.on_wait) if si and si.on_wait else []
    if len(waits) > _MAX_WAITS:
        drain_inst.ins.sync_info = mybir.SyncInfo(
            on_wait=waits[:_MAX_WAITS], on_update=list(si.on_update or [])
        )
        for i in range(_MAX_WAITS, len(waits), _MAX_WAITS):
            extra = self.nc.sync.drain()
            extra.ins.sync_info = mybir.SyncInfo(
                on_wait=waits[i:i + _MAX_WAITS], on_update=[]
            )

    self.nc.all_engine_barrier()
    assert self.sems is not None
    popped = self.nc._tile_sem_poison_stack.pop()
    assert popped is self._sem_poison
    self.nc.clear_and_free_semaphores(list(self.sems.allocated().values()))
    self.nc.all_engine_barrier()


tile.TileContext._drain_and_barrier = _patched_drain_and_barrier


# --- general BIR-level fix: split ANY instruction carrying more than one
# sem wait into single-wait Drain carriers + the original instruction with
# the last wait. Applied to the serialized BIR right before walrus.
def _split_multiwait_bir(ant_bir) -> bytes:
    import json as _json

    bir = _json.loads(ant_bir)
    for f in bir.get("functions", []):
        for blk in f.get("blocks", []):
            out = []
            for ins in blk.get("instructions", []):
                si = ins.get("sync_info")
                waits = (si or {}).get("on_wait") or []
                if len(waits) > 1:
                    for j, w in enumerate(waits[:-1]):
                        out.append({
                            "debug": ins.get("debug", 0),
                            "engine": ins["engine"],
                            "ins": [],
                            "name": f"{ins['name']}_w{j}",
                            "opcode": "Drain",
                            "outs": [],
                            "sync_info": {"on_update": [], "on_wait": [w]},
                        })
                    si["on_wait"] = [waits[-1]]
                out.append(ins)
            blk["instructions"] = out
    return _json.dumps(bir).encode()


def _install_bir_splitter():
    import concourse.bass_utils as _bu
    import concourse.bass2jax as _b2j

    orig = _bu.compile_bir_kernel
    if getattr(orig, "_multiwait_patched", False):
        return

    def patched(ant_bir_str, compile_dir_path, neff_name="file.neff", **kw):
        return orig(_split_multiwait_bir(ant_bir_str), compile_dir_path,
                    neff_name=neff_name, **kw)

    patched._multiwait_patched = True
    _bu.compile_bir_kernel = patched
    if hasattr(_b2j, "compile_bir_kernel"):
        _b2j.compile_bir_kernel = patched


_install_bir_splitter()


# --- custom fused DVE ops, registered at runtime (shas computed on the fly).
def _lse_ref(in0, in1, s0, s1, imm2):
    a = np.asarray(in0, np.float32)
    b = np.asarray(in1, np.float32)
    m = np.maximum(a, b)
    t = m - np.minimum(a, b)
    return (m + np.maximum(s0 + s1 * t, 0.0) ** 2).astype(np.float32)


def _injlp_ref(in0, in1, s0, s1, imm2):
    a = np.asarray(in0, np.float32)
    lp = np.asarray(in1, np.float32)
    k = np.arange(a.shape[-1], dtype=np.float32)[None, :]
    u = k - (s0 if isinstance(s0, float) else np.asarray(s0, np.float32))
    p = u * (u - 1.0)
    inj = np.minimum(p, 1.0) * imm2
    return (np.maximum(a, inj) + lp).astype(np.float32)


_OPS = None


def _make_ops():
    global _OPS
    if _OPS is not None:
        return _OPS
    from concourse import dve_ops as dops
    from concourse.dve_spec import (Spec, Src0, Src1, C0, C1, One,
                                    relu, sq, maxx, minn, lower)
    from concourse.dve_spec import _has_src1
    from concourse.dve_uop import DveOpSpec

    def register(name, body, ref):
        spec = Spec(body=body, reference=ref)
        row = dops._CUSTOM_DVE_ROW_BASE + len(dops.OPS)
        shas = {}
        for ver in ("v3", "v4"):
            uops = lower(spec, ver=ver)
            tmp = DveOpSpec(name=name, opcode=row, uops=uops,
                            rd1_en=_has_src1(spec))
            shas[ver] = tmp.sha(ver)
        op = dops.DveOp(name, spec, subdim=False, uops_sha=shas)
        dops.OPS.append(op)
        dops._SUB_OPCODE_FOR_NAME[name] = row
        dops.CUSTOM_DVE_SPECS[name] = spec
        return op

    m = maxx(Src0, Src1)
    n = minn(Src0, Src1)
    lse_body = m + sq(relu(C0 + C1 * (m - n)))
    lse_op = register("LSE_QSP_ANT", lse_body, _lse_ref)

    from concourse.dve_spec import Idx, C2
    u = Idx - C0
    p = u * (u - One)
    inj_body = maxx(Src0, minn(p, One) * C2) + Src1
    inj_op = register("INJLP_ANT", inj_body, _injlp_ref)

    _OPS = (lse_op, inj_op)
    return _OPS


_cached_nc = None


def build_bass():
    lse_op, inj_op = _make_ops()
    nc = bass.Bass()
    lp_d = nc.declare_dram_parameter("lp", [BL, NSTEP * TW], BF16, isOutput=False)
    lsk_d = nc.declare_dram_parameter("lsk", [BL, TW], F32, isOutput=False)
    x0_d = nc.declare_dram_parameter("x0", [BL, TW], F32, isOutput=False)
    cl_d = nc.declare_dram_parameter("cl", [BL, NSTEP], F32, isOutput=False)
    out_d = nc.declare_dram_parameter("out", [BL, 1], F32, isOutput=True)

    with tile.TileContext(nc) as tc:
        with (
            tc.tile_pool(name="lpp", bufs=1) as lp_pool,
            tc.tile_pool(name="persist", bufs=1) as pp,
        ):
            x_a = pp.tile([BL, TW], F32, tag="x_a")
            x_b = pp.tile([BL, TW], F32, tag="x_b")
            a2x = pp.tile([BL, TW], F32, tag="a2x")
            l1 = pp.tile([BL, TW], F32, tag="l1")
            l2 = pp.tile([BL, TW], F32, tag="l2")
            lskt = pp.tile([BL, TW], F32, tag="lskt")
            clt = pp.tile([BL, NSTEP], F32, tag="clt")
            # readout scratch (NEG-padded QSP LSE tree)
            am = pp.tile([BL, 136], F32, tag="am")
            sc = pp.tile([BL, 176], F32, tag="sc")
            loss = pp.tile([BL, 1], F32, tag="loss")

            nc.vector.memset(x_b[:, :], NEG)
            nc.vector.memset(am[:, :], NEG)
            nc.vector.memset(sc[:, :], NEG)
            nc.sync.dma_start(out=x_a[:, :], in_=x0_d[:, :])
            nc.sync.dma_start(out=lskt[:, :], in_=lsk_d[:, :])
            nc.sync.dma_start(out=clt[:, :], in_=cl_d[:, :])
            # all lp chunks issued upfront; arrivals stay ahead of the loop
            lpts = []
            lo = 0
            for ci, csz in enumerate(CHUNKS):
                lpt = lp_pool.tile([BL, csz * TW], BF16, tag=f"lp{ci}")
                nc.sync.dma_start(out=lpt[:, :],
                                  in_=lp_d[:, lo * TW:(lo + csz) * TW])
                lpts.append((lpt, lo, csz))
                lo += csz

            xc, xn = x_a, x_b
            for lpt, lo, csz in lpts:
                for il in range(csz):
                    i = lo + il
                    nc.vector.tensor_add(a2x[:, 2:2 + W], xc[:, 0:W],
                                         lskt[:, 2:2 + W])
                    nc.vector._custom_dve(lse_op, out=l1[:, 2:2 + W],
                                          in0=xc[:, 2:2 + W], in1=xc[:, 1:1 + W],
                                          s0=SP_C0, s1=SP_C1)
                    nc.vector._custom_dve(lse_op, out=l2[:, 2:2 + W],
                                          in0=l1[:, 2:2 + W], in1=a2x[:, 2:2 + W],
                                          s0=SP_C0, s1=SP_C1)
                    nc.vector._custom_dve(inj_op, out=xn[:, 2:2 + W],
                                          in0=l2[:, 2:2 + W],
                                          in1=lpt[:, il * TW + 2: il * TW + 2 + W],
                                          s0=clt[:, i:i + 1], s1=0.0, imm2=INJ_BIG)
                    xc, xn = xn, xc

            # readout: loss = -LSE_s(alpha_255[s] + beta_255[s])
            # alpha at cols 2..130 (s=0..128), beta at cols 261..133 (reversed).
            # LSE over 129 values as a NEG-padded binary tree of QSP ops
            # (widths 129-65-33-17-9-5-3-2-1), all on DVE: no act tables.
            nc.vector.tensor_add(am[:, 0:S], xc[:, FO:FO + S],
                                 xc[:, 261:132:-1])

            def tree(op, out_t, out_o, in_t, in_o, wlo, whi):
                nc.vector._custom_dve(
                    op, out=out_t[:, out_o:out_o + wlo],
                    in0=in_t[:, in_o:in_o + wlo],
                    in1=in_t[:, in_o + wlo:in_o + wlo + wlo],
                    s0=SP_C0, s1=SP_C1)

            tree(lse_op, sc, 0, am, 0, 65, 129)     # 129 -> 65   (am[129]=NEG)
            tree(lse_op, sc, 80, sc, 0, 33, 65)     # 65  -> 33   (sc[65]=NEG)
            tree(lse_op, sc, 120, sc, 80, 17, 33)   # 33  -> 17   (sc[113]=NEG)
            tree(lse_op, sc, 140, sc, 120, 9, 17)   # 17  -> 9    (sc[137]=NEG)
            tree(lse_op, sc, 152, sc, 140, 5, 9)    # 9   -> 5    (sc[149]=NEG)
            tree(lse_op, sc, 160, sc, 152, 3, 5)    # 5   -> 3    (sc[157]=NEG)
            tree(lse_op, sc, 168, sc, 160, 2, 3)    # 3   -> 2    (sc[163]=NEG)
            tree(lse_op, sc, 172, sc, 168, 1, 2)    # 2   -> 1
            nc.vector.tensor_scalar_mul(loss[:, 0:1], sc[:, 172:173], -1.0)
            nc.sync.dma_start(out=out_d[:, :], in_=loss[:, 0:1])
    # Raw Bass skips the InstISA byte-encoding pass (Bacc.compile runs it);
    # without it the NEFF compiler sees empty .instr -> "ISA wrong length".
    mybir.codegen_inst_isa_subclasses(nc)
    return nc


def _host_prep(y_pred, labels, input_length, label_length):
    blank = C - 1
    ext = np.full((B, S), blank, np.int32)
    ext[:, 1::2] = labels
    prev2 = np.concatenate([np.full((B, 2), -1, np.int32), ext[:, :-2]], axis=1)
    skip = (ext != blank) & (ext != prev2)                      # [B, S]

    q = np.take_along_axis(y_pred, ext[:, None, :], axis=2)     # [B, T, S]
    lp = np.log(q.astype(np.float32) + EPS)
    frozen = np.arange(T)[None, :] >= input_length[:, None]     # [B, T]
    lp[frozen, :] = 0.0

    lsk = np.where(skip, 0.0, NEG).astype(np.float32)           # [B, S]

    sellog = np.full((B, S), NEG, np.float32)
    s_last = 2 * label_length.astype(np.int64)                  # [B]
    np.put_along_axis(sellog, s_last[:, None], 0.0, axis=1)
    np.put_along_axis(sellog, (s_last - 1)[:, None], 0.0, axis=1)
    lens = input_length.astype(np.int64)

    # combined lp stream: fwd lp[i] at cols 2..130, bwd lp[510-i] reversed
    # at cols 133..261 (i=255 bwd part = 0 so the final bwd step yields
    # beta_255 without an lp add); NEG elsewhere so pad cols keep sinking.
    lpc = np.full((B, NSTEP, TW), NEG, np.float32)
    lpc[:, :, FO:FO + S] = lp[:, 0:NSTEP, :]
    lpc[:, 0:NSTEP - 1, BO:BO + S] = lp[:, 510:255:-1, ::-1]
    lpc[:, NSTEP - 1, BO:BO + S] = 0.0
    lpc = lpc.reshape(B, NSTEP * TW).astype(ml_dtypes.bfloat16)

    # combined skip gate: a2x[c] = X[c-2] + lskC[c]
    lskc = np.full((B, TW), NEG, np.float32)
    lskc[:, FO:FO + S] = lsk                                    # fwd: lsk[c-2]
    lskc[:, 135:262] = lsk[:, 2:S][:, ::-1]                     # bwd: lsk[263-c]

    # initial state: fwd alpha seed (0 at s=0), bwd g_511 = sellog + lp_511
    # for len==512 samples (reversed layout), NEG elsewhere.
    x0 = np.full((B, TW), NEG, np.float32)
    x0[:, FO] = 0.0
    g511 = np.where((lens == 512)[:, None], sellog + lp[:, 511, :], NEG)
    x0[:, BO:BO + S] = g511[:, ::-1].astype(np.float32)

    # injection column table: at step i = 511-len, window base Idx =
    # (col of s_last) - 2 = 259 - s_last; 9999 = no injection.
    cl = np.full((B, NSTEP), CINJ_OFF, np.float32)
    ii = 511 - lens                                             # [B]
    has = (ii >= 0) & (ii <= 255)
    bi = np.nonzero(has)[0]
    cl[bi, ii[bi]] = (259 - s_last[bi]).astype(np.float32)

    return lpc, lskc, x0, cl


def kernel(y_pred, labels, input_length, label_length):
    global _cached_nc
    y_pred = np.asarray(y_pred, np.float32)
    labels = np.asarray(labels, np.int32)
    input_length = np.asarray(input_length, np.int32)
    label_length = np.asarray(label_length, np.int32)
    lpc, lskc, x0, cl = _host_prep(y_pred, labels, input_length, label_length)
    if _cached_nc is None:
        _cached_nc = build_bass()
    in_maps = []
    for i in range(NCORES):
        sl = slice(i * BL, (i + 1) * BL)
        in_maps.append({"lp": lpc[sl], "lsk": lskc[sl], "x0": x0[sl],
                        "cl": cl[sl]})
    res = run_bass_kernel_spmd(_cached_nc, in_maps, list(range(NCORES)))
    out = np.concatenate([res.results[i]["out"] for i in range(NCORES)], axis=0)
    return out.astype(np.float32)


# revision 8
# speedup vs baseline: 1.1170x; 1.1170x over previous
"""CTC batch loss on 8 TRN2 NeuronCores — v7: parity-split merged chains.

Like v6 (fwd alpha + bwd beta chains meeting at t*=255, QSP-LSE custom DVE
ops, fused inject+emission op), but the extended-state row is split by
parity: blank states (even s) never take the s-2 skip path, so they need
only an LSE2 + emission (2 instructions over 132 cols) while labels
(odd s) run the full LSE3 path (4 instructions over 131 cols). Total
per-step DVE elements drop from 4x261=1044 to 4x131+2x132=788.

Layout (state row, width 268):
  cols 0,1   pad NEG
  cols 2..65    fwd labels l_k  (k=0..63, s=2k+1)
  cols 66..68   pad
  cols 69..132  bwd labels (reversed): gl_k at col 132-k
  cols 133,134  pad (never written)
  cols 135..199 fwd blanks b_k  (k=0..64, s=2k)
  cols 200,201  pad
  cols 202..266 bwd blanks (reversed): gb_k at col 266-k
  col 267    pad

Recurrences (g = beta + lp for the bwd chain, all QSP-approximated):
  fwd: l_k' = lp_l + LSE3(l_k, b_k, l_{k-1}*rep_k);  b_k' = lp_b + LSE2(b_k, l_{k-1})
  bwd: gl_k' = lp_l + LSE3(gl_k, gb_{k+1}, gl_{k+1}*rep_{k+1});  gb_k' = lp_b + LSE2(gb_k, gl_k)
Both halves of each group share one instruction window; the reversed bwd
layout makes all relative offsets match the fwd ones.
"""
import sys

for _p in ("/opt/trn_rl_repo", "/opt/pypackages"):
    if _p not in sys.path:
        sys.path.insert(0, _p)

import numpy as np
import ml_dtypes

import concourse.bass as bass
import concourse.tile as tile
from concourse import mybir
from concourse.bass_utils import run_bass_kernel_spmd

B, T, C, L = 1024, 512, 128, 64
S = 2 * L + 1
NCORES = 8
BL = B // NCORES
EPS = 1e-7
NEG = -30000.0

TW = 268               # state row width
LF = 2                 # fwd label k at col LF+k       (2..65)
LB = 69                # bwd label k at col 132-k      (69..132)
BF = 135               # fwd blank k at col BF+k       (135..199)
BB = 202               # bwd blank k at col 266-k      (202..266)
WL = 131               # label instruction window: out cols [2, 133)
WB = 132               # blank instruction window: out cols [135, 267)
LPW = 132              # per-step lp stream stride (both groups)
NSTEP = 256
CHUNKS = [4, 12, 16, 32, 32, 32, 32, 32, 32, 32]
assert sum(CHUNKS) == NSTEP
CINJ_OFF = 9999.0

F32 = mybir.dt.float32
BF16 = mybir.dt.bfloat16
ALU = mybir.AluOpType

SP_C0 = 0.8129
SP_C1 = -0.2261
INJ_BIG = -3.0e38

_MAX_WAITS = 1


def _patched_drain_and_barrier(self, tick_clock, wait_clock):
    from concourse.vector_clock import ScopedClock

    drain_inst = self.nc.sync.drain()
    wait_clock.add_sem_waits(
        drain_inst.ins, ScopedClock({None: tick_clock.global_clock})
    )
    si = drain_inst.ins.sync_info
    waits = list(si.on_wait) if si and si.on_wait else []
    if len(waits) > _MAX_WAITS:
        drain_inst.ins.sync_info = mybir.SyncInfo(
            on_wait=waits[:_MAX_WAITS], on_update=list(si.on_update or [])
        )
        for i in range(_MAX_WAITS, len(waits), _MAX_WAITS):
            extra = self.nc.sync.drain()
            extra.ins.sync_info = mybir.SyncInfo(
                on_wait=waits[i:i + _MAX_WAITS], on_update=[]
            )

    self.nc.all_engine_barrier()
    assert self.sems is not None
    popped = self.nc._tile_sem_poison_stack.pop()
    assert popped is self._sem_poison
    self.nc.clear_and_free_semaphores(list(self.sems.allocated().values()))
    self.nc.all_engine_barrier()


tile.TileContext._drain_and_barrier = _patched_drain_and_barrier


def _split_multiwait_bir(ant_bir) -> bytes:
    import json as _json

    bir = _json.loads(ant_bir)
    for f in bir.get("functions", []):
        for blk in f.get("blocks", []):
            out = []
            for ins in blk.get("instructions", []):
                si = ins.get("sync_info")
                waits = (si or {}).get("on_wait") or []
                if len(waits) > 1:
                    for j, w in enumerate(waits[:-1]):
                        out.append({
                            "debug": ins.get("debug", 0),
                            "engine": ins["engine"],
                            "ins": [],
                            "name": f"{ins['name']}_w{j}",
                            "opcode": "Drain",
                            "outs": [],
                            "sync_info": {"on_update": [], "on_wait": [w]},
                        })
                    si["on_wait"] = [waits[-1]]
                out.append(ins)
            blk["instructions"] = out
    return _json.dumps(bir).encode()


def _install_bir_splitter():
    import concourse.bass_utils as _bu
    import concourse.bass2jax as _b2j

    orig = _bu.compile_bir_kernel
    if getattr(orig, "_multiwait_patched", False):
        return

    def patched(ant_bir_str, compile_dir_path, neff_name="file.neff", **kw):
        return orig(_split_multiwait_bir(ant_bir_str), compile_dir_path,
                    neff_name=neff_name, **kw)

    patched._multiwait_patched = True
    _bu.compile_bir_kernel = patched
    if hasattr(_b2j, "compile_bir_kernel"):
        _b2j.compile_bir_kernel = patched


_install_bir_splitter()


def _lse_ref(in0, in1, s0, s1, imm2):
    a = np.asarray(in0, np.float32)
    b = np.asarray(in1, np.float32)
    m = np.maximum(a, b)
    t = m - np.minimum(a, b)
    return (m + np.maximum(s0 + s1 * t, 0.0) ** 2).astype(np.float32)


def _inj1_ref(in0, in1, s0, s1, imm2):
    a = np.asarray(in0, np.float32)
    lp = np.asarray(in1, np.float32)
    k = np.arange(a.shape[-1], dtype=np.float32)[None, :]
    u = k - (s0 if isinstance(s0, float) else np.asarray(s0, np.float32))
    inj = np.minimum(u * u, 1.0) * (s1 if isinstance(s1, float)
                                    else np.asarray(s1, np.float32))
    return (np.maximum(a, inj) + lp).astype(np.float32)


_OPS = None


def _make_ops():
    global _OPS
    if _OPS is not None:
        return _OPS
    from concourse import dve_ops as dops
    from concourse.dve_spec import (Spec, Src0, Src1, C0, C1, One, Idx,
                                    relu, sq, maxx, minn, lower)
    from concourse.dve_spec import _has_src1
    from concourse.dve_uop import DveOpSpec

    def register(name, body, ref):
        for existing in dops.OPS:
            if existing.name == name:
                return existing
        spec = Spec(body=body, reference=ref)
        row = dops._CUSTOM_DVE_ROW_BASE + len(dops.OPS)
        shas = {}
        for ver in ("v3", "v4"):
            uops = lower(spec, ver=ver)
            tmp = DveOpSpec(name=name, opcode=row, uops=uops,
                            rd1_en=_has_src1(spec))
            shas[ver] = tmp.sha(ver)
        op = dops.DveOp(name, spec, subdim=False, uops_sha=shas)
        dops.OPS.append(op)
        dops._SUB_OPCODE_FOR_NAME[name] = row
        dops.CUSTOM_DVE_SPECS[name] = spec
        return op

    m = maxx(Src0, Src1)
    n = minn(Src0, Src1)
    lse_body = m + sq(relu(C0 + C1 * (m - n)))
    lse_op = register("LSE_QSP_ANT", lse_body, _lse_ref)

    # single-column inject + emission add: out = max(Src0, w) + Src1 where
    # w = 0.0 exactly at Idx==C0 and C1=-3e38 elsewhere (C0=9999: no-op).
    u = Idx - C0
    inj1_body = maxx(Src0, minn(sq(u), One) * C1) + Src1
    inj1_op = register("INJ1_ANT", inj1_body, _inj1_ref)

    _OPS = (lse_op, inj1_op)
    return _OPS


_cached_nc = None


def build_bass():
    lse_op, inj1_op = _make_ops()
    nc = bass.Bass()
    lpl_d = nc.declare_dram_parameter("lpl", [BL, NSTEP * LPW], BF16, isOutput=False)
    lpb_d = nc.declare_dram_parameter("lpb", [BL, NSTEP * LPW], BF16, isOutput=False)
    rep_d = nc.declare_dram_parameter("rep", [BL, TW], F32, isOutput=False)
    x0_d = nc.declare_dram_parameter("x0", [BL, TW], F32, isOutput=False)
    cll_d = nc.declare_dram_parameter("cll", [BL, NSTEP], F32, isOutput=False)
    clb_d = nc.declare_dram_parameter("clb", [BL, NSTEP], F32, isOutput=False)
    out_d = nc.declare_dram_parameter("out", [BL, 1], F32, isOutput=True)

    with tile.TileContext(nc) as tc:
        with (
            tc.tile_pool(name="lpp", bufs=1) as lp_pool,
            tc.tile_pool(name="persist", bufs=1) as pp,
        ):
            x_a = pp.tile([BL, TW], F32, tag="x_a")
            x_b = pp.tile([BL, TW], F32, tag="x_b")
            lrx = pp.tile([BL, TW], F32, tag="lrx")
            l1t = pp.tile([BL, TW], F32, tag="l1t")
            l2t = pp.tile([BL, TW], F32, tag="l2t")
            l1b = pp.tile([BL, TW], F32, tag="l1b")
            rept = pp.tile([BL, TW], F32, tag="rept")
            cllt = pp.tile([BL, NSTEP], F32, tag="cllt")
            clbt = pp.tile([BL, NSTEP], F32, tag="clbt")
            am = pp.tile([BL, 136], F32, tag="am")
            sc = pp.tile([BL, 176], F32, tag="sc")
            loss = pp.tile([BL, 1], F32, tag="loss")

            nc.vector.memset(x_b[:, :], NEG)
            nc.vector.memset(am[:, :], NEG)
            nc.vector.memset(sc[:, :], NEG)
            nc.sync.dma_start(out=x_a[:, :], in_=x0_d[:, :])
            nc.sync.dma_start(out=rept[:, :], in_=rep_d[:, :])
            nc.sync.dma_start(out=cllt[:, :], in_=cll_d[:, :])
            nc.sync.dma_start(out=clbt[:, :], in_=clb_d[:, :])
            lpts = []
            lo = 0
            for ci, csz in enumerate(CHUNKS):
                lplt = lp_pool.tile([BL, csz * LPW], BF16, tag=f"lpl{ci}")
                nc.sync.dma_start(out=lplt[:, :],
                                  in_=lpl_d[:, lo * LPW:(lo + csz) * LPW])
                lpbt = lp_pool.tile([BL, csz * LPW], BF16, tag=f"lpb{ci}")
                nc.sync.dma_start(out=lpbt[:, :],
                                  in_=lpb_d[:, lo * LPW:(lo + csz) * LPW])
                lpts.append((lplt, lpbt, lo, csz))
                lo += csz

            xc, xn = x_a, x_b
            for lplt, lpbt, lo, csz in lpts:
                for il in range(csz):
                    i = lo + il
                    # labels: out cols [2,133)
                    nc.vector.tensor_add(lrx[:, 2:2 + WL], xc[:, 1:1 + WL],
                                         rept[:, 2:2 + WL])
                    nc.vector._custom_dve(lse_op, out=l1t[:, 2:2 + WL],
                                          in0=xc[:, 2:2 + WL],
                                          in1=xc[:, BF:BF + WL],
                                          s0=SP_C0, s1=SP_C1)
                    nc.vector._custom_dve(lse_op, out=l2t[:, 2:2 + WL],
                                          in0=l1t[:, 2:2 + WL],
                                          in1=lrx[:, 2:2 + WL],
                                          s0=SP_C0, s1=SP_C1)
                    nc.vector._custom_dve(inj1_op, out=xn[:, 2:2 + WL],
                                          in0=l2t[:, 2:2 + WL],
                                          in1=lplt[:, il * LPW: il * LPW + WL],
                                          s0=cllt[:, i:i + 1], s1=INJ_BIG)
                    # blanks: out cols [135,267)
                    nc.vector._custom_dve(lse_op, out=l1b[:, BF:BF + WB],
                                          in0=xc[:, BF:BF + WB],
                                          in1=xc[:, 1:1 + WB],
                                          s0=SP_C0, s1=SP_C1)
                    nc.vector._custom_dve(inj1_op, out=xn[:, BF:BF + WB],
                                          in0=l1b[:, BF:BF + WB],
                                          in1=lpbt[:, il * LPW: il * LPW + WB],
                                          s0=clbt[:, i:i + 1], s1=INJ_BIG)
                    xc, xn = xn, xc

            # readout: am[0:64]=labels alpha+beta, am[64:129]=blanks
            nc.vector.tensor_add(am[:, 0:64], xc[:, LF:LF + 64],
                                 xc[:, 132:68:-1])
            nc.vector.tensor_add(am[:, 64:129], xc[:, BF:BF + 65],
                                 xc[:, 266:201:-1])

            def tree(out_o, in_t, in_o, wlo):
                nc.vector._custom_dve(
                    lse_op, out=sc[:, out_o:out_o + wlo],
                    in0=in_t[:, in_o:in_o + wlo],
                    in1=in_t[:, in_o + wlo:in_o + 2 * wlo],
                    s0=SP_C0, s1=SP_C1)

            tree(0, am, 0, 65)      # 129 -> 65  (am[129]=NEG)
            tree(80, sc, 0, 33)     # 65 -> 33   (sc[65]=NEG)
            tree(120, sc, 80, 17)   # 33 -> 17   (sc[113]=NEG)
            tree(140, sc, 120, 9)   # 17 -> 9    (sc[137]=NEG)
            tree(152, sc, 140, 5)   # 9 -> 5     (sc[149]=NEG)
            tree(160, sc, 152, 3)   # 5 -> 3     (sc[157]=NEG)
            tree(168, sc, 160, 2)   # 3 -> 2     (sc[163]=NEG)
            tree(172, sc, 168, 1)   # 2 -> 1
            nc.vector.tensor_scalar_mul(loss[:, 0:1], sc[:, 172:173], -1.0)
            nc.sync.dma_start(out=out_d[:, :], in_=loss[:, 0:1])
    mybir.codegen_inst_isa_subclasses(nc)
    return nc


def _host_prep(y_pred, labels, input_length, label_length):
    blank = C - 1
    lab = labels.astype(np.int64)
    q_l = np.take_along_axis(y_pred, lab[:, None, :], axis=2)   # [B,T,64]
    lp_l = np.log(q_l.astype(np.float32) + EPS)                 # label lp
    lp_b = np.log(y_pred[:, :, blank].astype(np.float32) + EPS) # [B,T] blank lp
    frozen = np.arange(T)[None, :] >= input_length[:, None]
    lp_l[frozen, :] = 0.0
    lp_b[frozen] = 0.0

    # rep gate for label k vs k-1 (fwd: gates l_{k-1} -> l_k)
    rep = np.full((B, L), 0.0, np.float32)
    rep[:, 1:] = np.where(labels[:, 1:] != labels[:, :-1], 0.0, NEG)
    # rep[:, 0] = 0.0: the gated read hits the NEG pad anyway.

    lens = input_length.astype(np.int64)
    llen = label_length.astype(np.int64)                        # [B] in [32,64]

    # combined lp streams (NEG on pads so they keep sinking)
    lpl = np.full((B, NSTEP, LPW), NEG, np.float32)
    lpl[:, :, 0:64] = lp_l[:, 0:NSTEP, :]
    # bwd: j=67..130 <-> label k=130-j, lp[510-i]
    lpl[:, 0:NSTEP - 1, 67:131] = lp_l[:, 510:255:-1, ::-1]
    lpl[:, NSTEP - 1, 67:131] = 0.0
    lpl = lpl.reshape(B, NSTEP * LPW).astype(ml_dtypes.bfloat16)

    lpb = np.full((B, NSTEP, LPW), NEG, np.float32)
    lpb[:, :, 0:65] = lp_b[:, 0:NSTEP, None]
    lpb[:, 0:NSTEP - 1, 67:132] = lp_b[:, 510:255:-1, None]
    lpb[:, NSTEP - 1, 67:132] = 0.0
    lpb = lpb.reshape(B, NSTEP * LPW).astype(ml_dtypes.bfloat16)

    # combined rep stream: fwd col 2+k; bwd col 69+r gated by rep_{64-r}
    repc = np.full((B, TW), NEG, np.float32)
    repc[:, 2:66] = rep
    repc[:, 70:133] = rep[:, 1:64][:, ::-1]   # r=1..63 -> rep[64-r]
    # col 69 (r=0) stays NEG: label 64 does not exist.

    # initial state
    x0 = np.full((B, TW), NEG, np.float32)
    x0[:, BF] = 0.0                                             # alpha seed s=0
    is512 = lens == 512
    # g511 = sellog + lp_511 (only for len==512 samples): states s_last=2*llen
    # (blank k=llen) and s_last-1 (label k=llen-1)
    bi = np.nonzero(is512)[0]
    x0[bi, 266 - llen[bi]] = lp_b[bi, 511]
    x0[bi, 133 - llen[bi]] = lp_l[bi, 511, llen[bi] - 1]

    # injection tables: at step i = 511-len, single-col windows
    cll = np.full((B, NSTEP), CINJ_OFF, np.float32)
    clb = np.full((B, NSTEP), CINJ_OFF, np.float32)
    ii = 511 - lens
    has = (ii >= 0) & (ii <= 255)
    bi = np.nonzero(has)[0]
    cll[bi, ii[bi]] = (131 - llen[bi]).astype(np.float32)  # label Idx=131-llen
    clb[bi, ii[bi]] = (131 - llen[bi]).astype(np.float32)  # blank Idx=131-llen

    return lpl, lpb, repc, x0, cll, clb


def kernel(y_pred, labels, input_length, label_length):
    global _cached_nc
    y_pred = np.asarray(y_pred, np.float32)
    labels = np.asarray(labels, np.int32)
    input_length = np.asarray(input_length, np.int32)
    label_length = np.asarray(label_length, np.int32)
    lpl, lpb, repc, x0, cll, clb = _host_prep(
        y_pred, labels, input_length, label_length)
    if _cached_nc is None:
        _cached_nc = build_bass()
    in_maps = []
    for i in range(NCORES):
        sl = slice(i * BL, (i + 1) * BL)
        in_maps.append({"lpl": lpl[sl], "lpb": lpb[sl], "rep": repc[sl],
                        "x0": x0[sl], "cll": cll[sl], "clb": clb[sl]})
    res = run_bass_kernel_spmd(_cached_nc, in_maps, list(range(NCORES)))
    out = np.concatenate([res.results[i]["out"] for i in range(NCORES)], axis=0)
    return out.astype(np.float32)


# revision 9
# speedup vs baseline: 1.3371x; 1.1971x over previous
"""CTC batch loss on 8 TRN2 NeuronCores — v7: parity-split merged chains.

Like v6 (fwd alpha + bwd beta chains meeting at t*=255, QSP-LSE custom DVE
ops, fused inject+emission op), but the extended-state row is split by
parity: blank states (even s) never take the s-2 skip path, so they need
only an LSE2 + emission (2 instructions over 132 cols) while labels
(odd s) run the full LSE3 path (4 instructions over 131 cols). Total
per-step DVE elements drop from 4x261=1044 to 4x131+2x132=788.

Layout (state row, width 268):
  cols 0,1   pad NEG
  cols 2..65    fwd labels l_k  (k=0..63, s=2k+1)
  cols 66..68   pad
  cols 69..132  bwd labels (reversed): gl_k at col 132-k
  cols 133,134  pad (never written)
  cols 135..199 fwd blanks b_k  (k=0..64, s=2k)
  cols 200,201  pad
  cols 202..266 bwd blanks (reversed): gb_k at col 266-k
  col 267    pad

Recurrences (g = beta + lp for the bwd chain, all QSP-approximated):
  fwd: l_k' = lp_l + LSE3(l_k, b_k, l_{k-1}*rep_k);  b_k' = lp_b + LSE2(b_k, l_{k-1})
  bwd: gl_k' = lp_l + LSE3(gl_k, gb_{k+1}, gl_{k+1}*rep_{k+1});  gb_k' = lp_b + LSE2(gb_k, gl_k)
Both halves of each group share one instruction window; the reversed bwd
layout makes all relative offsets match the fwd ones.
"""
import sys

for _p in ("/opt/trn_rl_repo", "/opt/pypackages"):
    if _p not in sys.path:
        sys.path.insert(0, _p)

import numpy as np
import ml_dtypes

import concourse.bass as bass
import concourse.tile as tile
from concourse import mybir
from concourse.bass_utils import run_bass_kernel_spmd

B, T, C, L = 1024, 512, 128, 64
S = 2 * L + 1
NCORES = 8
BL = B // NCORES
EPS = 1e-7
NEG = -30000.0

TW = 268               # state row width
LF = 2                 # fwd label k at col LF+k       (2..65)
LB = 69                # bwd label k at col 132-k      (69..132)
BF = 135               # fwd blank k at col BF+k       (135..199)
BB = 202               # bwd blank k at col 266-k      (202..266)
WL = 131               # label instruction window: out cols [2, 133)
WB = 132               # blank instruction window: out cols [135, 267)
LPW = 132              # per-step lp stream stride (both groups)
NSTEP = 256
CHUNKS = [4, 12, 16, 32, 32, 32, 32, 32, 32, 32]
assert sum(CHUNKS) == NSTEP
CINJ_OFF = 9999.0

F32 = mybir.dt.float32
BF16 = mybir.dt.bfloat16
ALU = mybir.AluOpType

SP_C0 = 0.8129
SP_C1 = -0.2261
INJ_BIG = -3.0e38

_MAX_WAITS = 1


def _patched_drain_and_barrier(self, tick_clock, wait_clock):
    from concourse.vector_clock import ScopedClock

    drain_inst = self.nc.sync.drain()
    wait_clock.add_sem_waits(
        drain_inst.ins, ScopedClock({None: tick_clock.global_clock})
    )
    si = drain_inst.ins.sync_info
    waits = list(si.on_wait) if si and si.on_wait else []
    if len(waits) > _MAX_WAITS:
        drain_inst.ins.sync_info = mybir.SyncInfo(
            on_wait=waits[:_MAX_WAITS], on_update=list(si.on_update or [])
        )
        for i in range(_MAX_WAITS, len(waits), _MAX_WAITS):
            extra = self.nc.sync.drain()
            extra.ins.sync_info = mybir.SyncInfo(
                on_wait=waits[i:i + _MAX_WAITS], on_update=[]
            )

    self.nc.all_engine_barrier()
    assert self.sems is not None
    popped = self.nc._tile_sem_poison_stack.pop()
    assert popped is self._sem_poison
    self.nc.clear_and_free_semaphores(list(self.sems.allocated().values()))
    self.nc.all_engine_barrier()


tile.TileContext._drain_and_barrier = _patched_drain_and_barrier


def _split_multiwait_bir(ant_bir) -> bytes:
    import json as _json

    bir = _json.loads(ant_bir)
    for f in bir.get("functions", []):
        for blk in f.get("blocks", []):
            out = []
            for ins in blk.get("instructions", []):
                si = ins.get("sync_info")
                waits = (si or {}).get("on_wait") or []
                if len(waits) > 1:
                    for j, w in enumerate(waits[:-1]):
                        out.append({
                            "debug": ins.get("debug", 0),
                            "engine": ins["engine"],
                            "ins": [],
                            "name": f"{ins['name']}_w{j}",
                            "opcode": "Drain",
                            "outs": [],
                            "sync_info": {"on_update": [], "on_wait": [w]},
                        })
                    si["on_wait"] = [waits[-1]]
                out.append(ins)
            blk["instructions"] = out
    return _json.dumps(bir).encode()


def _install_bir_splitter():
    import concourse.bass_utils as _bu
    import concourse.bass2jax as _b2j

    orig = _bu.compile_bir_kernel
    if getattr(orig, "_multiwait_patched", False):
        return

    def patched(ant_bir_str, compile_dir_path, neff_name="file.neff", **kw):
        return orig(_split_multiwait_bir(ant_bir_str), compile_dir_path,
                    neff_name=neff_name, **kw)

    patched._multiwait_patched = True
    _bu.compile_bir_kernel = patched
    if hasattr(_b2j, "compile_bir_kernel"):
        _b2j.compile_bir_kernel = patched


_install_bir_splitter()


def _lse_ref(in0, in1, s0, s1, imm2):
    a = np.asarray(in0, np.float32)
    b = np.asarray(in1, np.float32)
    m = np.maximum(a, b)
    t = m - np.minimum(a, b)
    return (m + np.maximum(s0 + s1 * t, 0.0) ** 2).astype(np.float32)


def _inj1_ref(in0, in1, s0, s1, imm2):
    a = np.asarray(in0, np.float32)
    lp = np.asarray(in1, np.float32)
    k = np.arange(a.shape[-1], dtype=np.float32)[None, :]
    u = k - (s0 if isinstance(s0, float) else np.asarray(s0, np.float32))
    inj = np.minimum(u * u, 1.0) * (s1 if isinstance(s1, float)
                                    else np.asarray(s1, np.float32))
    return (np.maximum(a, inj) + lp).astype(np.float32)


_OPS = None


def _make_ops():
    global _OPS
    if _OPS is not None:
        return _OPS
    from concourse import dve_ops as dops
    from concourse.dve_spec import (Spec, Src0, Src1, C0, C1, One, Idx,
                                    relu, sq, maxx, minn, lower)
    from concourse.dve_spec import _has_src1
    from concourse.dve_uop import DveOpSpec

    def register(name, body, ref):
        for existing in dops.OPS:
            if existing.name == name:
                return existing
        spec = Spec(body=body, reference=ref)
        row = dops._CUSTOM_DVE_ROW_BASE + len(dops.OPS)
        shas = {}
        for ver in ("v3", "v4"):
            uops = lower(spec, ver=ver)
            tmp = DveOpSpec(name=name, opcode=row, uops=uops,
                            rd1_en=_has_src1(spec))
            shas[ver] = tmp.sha(ver)
        op = dops.DveOp(name, spec, subdim=False, uops_sha=shas)
        dops.OPS.append(op)
        dops._SUB_OPCODE_FOR_NAME[name] = row
        dops.CUSTOM_DVE_SPECS[name] = spec
        return op

    m = maxx(Src0, Src1)
    n = minn(Src0, Src1)
    lse_body = m + sq(relu(C0 + C1 * (m - n)))
    lse_op = register("LSE_QSP_ANT", lse_body, _lse_ref)

    # single-column inject + emission add: out = max(Src0, w) + Src1 where
    # w = 0.0 exactly at Idx==C0 and C1=-3e38 elsewhere (C0=9999: no-op).
    u = Idx - C0
    inj1_body = maxx(Src0, minn(sq(u), One) * C1) + Src1
    inj1_op = register("INJ1_ANT", inj1_body, _inj1_ref)

    _OPS = (lse_op, inj1_op)
    return _OPS


_cached_nc = None


def build_bass():
    lse_op, inj1_op = _make_ops()
    nc = bass.Bass()
    lpl_d = nc.declare_dram_parameter("lpl", [BL, NSTEP * LPW], BF16, isOutput=False)
    lpb_d = nc.declare_dram_parameter("lpb", [BL, NSTEP * LPW], BF16, isOutput=False)
    rep_d = nc.declare_dram_parameter("rep", [BL, TW], F32, isOutput=False)
    x0_d = nc.declare_dram_parameter("x0", [BL, TW], F32, isOutput=False)
    cll_d = nc.declare_dram_parameter("cll", [BL, NSTEP], F32, isOutput=False)
    clb_d = nc.declare_dram_parameter("clb", [BL, NSTEP], F32, isOutput=False)
    out_d = nc.declare_dram_parameter("out", [BL, 1], F32, isOutput=True)

    with tile.TileContext(nc) as tc:
        with (
            tc.tile_pool(name="lpp", bufs=1) as lp_pool,
            tc.tile_pool(name="persist", bufs=1) as pp,
        ):
            x_a = pp.tile([BL, TW], F32, tag="x_a")
            x_b = pp.tile([BL, TW], F32, tag="x_b")
            lrx_a = pp.tile([BL, TW], F32, tag="lrx_a")
            lrx_b = pp.tile([BL, TW], F32, tag="lrx_b")
            l1t = pp.tile([BL, TW], F32, tag="l1t")
            l2t = pp.tile([BL, TW], F32, tag="l2t")
            l1b = pp.tile([BL, TW], F32, tag="l1b")
            rept = pp.tile([BL, TW], F32, tag="rept")
            cllt = pp.tile([BL, NSTEP], F32, tag="cllt")
            clbt = pp.tile([BL, NSTEP], F32, tag="clbt")
            am = pp.tile([BL, 136], F32, tag="am")
            sc = pp.tile([BL, 176], F32, tag="sc")
            loss = pp.tile([BL, 1], F32, tag="loss")

            nc.vector.memset(x_b[:, :], NEG)
            nc.vector.memset(am[:, :], NEG)
            nc.vector.memset(sc[:, :], NEG)
            nc.sync.dma_start(out=x_a[:, :], in_=x0_d[:, :])
            nc.sync.dma_start(out=rept[:, :], in_=rep_d[:, :])
            nc.sync.dma_start(out=cllt[:, :], in_=cll_d[:, :])
            nc.sync.dma_start(out=clbt[:, :], in_=clb_d[:, :])
            lpts = []
            lo = 0
            for ci, csz in enumerate(CHUNKS):
                lplt = lp_pool.tile([BL, csz * LPW], BF16, tag=f"lpl{ci}")
                nc.sync.dma_start(out=lplt[:, :],
                                  in_=lpl_d[:, lo * LPW:(lo + csz) * LPW])
                lpbt = lp_pool.tile([BL, csz * LPW], BF16, tag=f"lpb{ci}")
                nc.sync.dma_start(out=lpbt[:, :],
                                  in_=lpb_d[:, lo * LPW:(lo + csz) * LPW])
                lpts.append((lplt, lpbt, lo, csz))
                lo += csz

            xc, xn = x_a, x_b
            lrc, lrn = lrx_a, lrx_b
            for lplt, lpbt, lo, csz in lpts:
                for il in range(csz):
                    i = lo + il
                    # gated label skip path on GPSIMD: runs in the shadow of
                    # the DVE ops below (consumed only by the 4th DVE op).
                    nc.gpsimd.tensor_add(lrc[:, 2:2 + WL], xc[:, 1:1 + WL],
                                         rept[:, 2:2 + WL])
                    # DVE: label LSE2, blank pipeline, then label LSE3 tail
                    nc.vector._custom_dve(lse_op, out=l1t[:, 2:2 + WL],
                                          in0=xc[:, 2:2 + WL],
                                          in1=xc[:, BF:BF + WL],
                                          s0=SP_C0, s1=SP_C1)
                    nc.vector._custom_dve(lse_op, out=l1b[:, BF:BF + WB],
                                          in0=xc[:, BF:BF + WB],
                                          in1=xc[:, 1:1 + WB],
                                          s0=SP_C0, s1=SP_C1)
                    nc.vector._custom_dve(inj1_op, out=xn[:, BF:BF + WB],
                                          in0=l1b[:, BF:BF + WB],
                                          in1=lpbt[:, il * LPW: il * LPW + WB],
                                          s0=clbt[:, i:i + 1], s1=INJ_BIG)
                    nc.vector._custom_dve(lse_op, out=l2t[:, 2:2 + WL],
                                          in0=l1t[:, 2:2 + WL],
                                          in1=lrc[:, 2:2 + WL],
                                          s0=SP_C0, s1=SP_C1)
                    nc.vector._custom_dve(inj1_op, out=xn[:, 2:2 + WL],
                                          in0=l2t[:, 2:2 + WL],
                                          in1=lplt[:, il * LPW: il * LPW + WL],
                                          s0=cllt[:, i:i + 1], s1=INJ_BIG)
                    xc, xn = xn, xc
                    lrc, lrn = lrn, lrc

            # readout: am[0:64]=labels alpha+beta, am[64:129]=blanks
            nc.vector.tensor_add(am[:, 0:64], xc[:, LF:LF + 64],
                                 xc[:, 132:68:-1])
            nc.vector.tensor_add(am[:, 64:129], xc[:, BF:BF + 65],
                                 xc[:, 266:201:-1])

            def tree(out_o, in_t, in_o, wlo):
                nc.vector._custom_dve(
                    lse_op, out=sc[:, out_o:out_o + wlo],
                    in0=in_t[:, in_o:in_o + wlo],
                    in1=in_t[:, in_o + wlo:in_o + 2 * wlo],
                    s0=SP_C0, s1=SP_C1)

            tree(0, am, 0, 65)      # 129 -> 65  (am[129]=NEG)
            tree(80, sc, 0, 33)     # 65 -> 33   (sc[65]=NEG)
            tree(120, sc, 80, 17)   # 33 -> 17   (sc[113]=NEG)
            tree(140, sc, 120, 9)   # 17 -> 9    (sc[137]=NEG)
            tree(152, sc, 140, 5)   # 9 -> 5     (sc[149]=NEG)
            tree(160, sc, 152, 3)   # 5 -> 3     (sc[157]=NEG)
            tree(168, sc, 160, 2)   # 3 -> 2     (sc[163]=NEG)
            tree(172, sc, 168, 1)   # 2 -> 1
            nc.vector.tensor_scalar_mul(loss[:, 0:1], sc[:, 172:173], -1.0)
            nc.sync.dma_start(out=out_d[:, :], in_=loss[:, 0:1])
    mybir.codegen_inst_isa_subclasses(nc)
    return nc


def _host_prep(y_pred, labels, input_length, label_length):
    blank = C - 1
    lab = labels.astype(np.int64)
    q_l = np.take_along_axis(y_pred, lab[:, None, :], axis=2)   # [B,T,64]
    lp_l = np.log(q_l.astype(np.float32) + EPS)                 # label lp
    lp_b = np.log(y_pred[:, :, blank].astype(np.float32) + EPS) # [B,T] blank lp
    frozen = np.arange(T)[None, :] >= input_length[:, None]
    lp_l[frozen, :] = 0.0
    lp_b[frozen] = 0.0

    # rep gate for label k vs k-1 (fwd: gates l_{k-1} -> l_k)
    rep = np.full((B, L), 0.0, np.float32)
    rep[:, 1:] = np.where(labels[:, 1:] != labels[:, :-1], 0.0, NEG)
    # rep[:, 0] = 0.0: the gated read hits the NEG pad anyway.

    lens = input_length.astype(np.int64)
    llen = label_length.astype(np.int64)                        # [B] in [32,64]

    # combined lp streams (NEG on pads so they keep sinking)
    lpl = np.full((B, NSTEP, LPW), NEG, np.float32)
    lpl[:, :, 0:64] = lp_l[:, 0:NSTEP, :]
    # bwd: j=67..130 <-> label k=130-j, lp[510-i]
    lpl[:, 0:NSTEP - 1, 67:131] = lp_l[:, 510:255:-1, ::-1]
    lpl[:, NSTEP - 1, 67:131] = 0.0
    lpl = lpl.reshape(B, NSTEP * LPW).astype(ml_dtypes.bfloat16)

    lpb = np.full((B, NSTEP, LPW), NEG, np.float32)
    lpb[:, :, 0:65] = lp_b[:, 0:NSTEP, None]
    lpb[:, 0:NSTEP - 1, 67:132] = lp_b[:, 510:255:-1, None]
    lpb[:, NSTEP - 1, 67:132] = 0.0
    lpb = lpb.reshape(B, NSTEP * LPW).astype(ml_dtypes.bfloat16)

    # combined rep stream: fwd col 2+k; bwd col 69+r gated by rep_{64-r}
    repc = np.full((B, TW), NEG, np.float32)
    repc[:, 2:66] = rep
    repc[:, 70:133] = rep[:, 1:64][:, ::-1]   # r=1..63 -> rep[64-r]
    # col 69 (r=0) stays NEG: label 64 does not exist.

    # initial state
    x0 = np.full((B, TW), NEG, np.float32)
    x0[:, BF] = 0.0                                             # alpha seed s=0
    is512 = lens == 512
    # g511 = sellog + lp_511 (only for len==512 samples): states s_last=2*llen
    # (blank k=llen) and s_last-1 (label k=llen-1)
    bi = np.nonzero(is512)[0]
    x0[bi, 266 - llen[bi]] = lp_b[bi, 511]
    x0[bi, 133 - llen[bi]] = lp_l[bi, 511, llen[bi] - 1]

    # injection tables: at step i = 511-len, single-col windows
    cll = np.full((B, NSTEP), CINJ_OFF, np.float32)
    clb = np.full((B, NSTEP), CINJ_OFF, np.float32)
    ii = 511 - lens
    has = (ii >= 0) & (ii <= 255)
    bi = np.nonzero(has)[0]
    cll[bi, ii[bi]] = (131 - llen[bi]).astype(np.float32)  # label Idx=131-llen
    clb[bi, ii[bi]] = (131 - llen[bi]).astype(np.float32)  # blank Idx=131-llen

    return lpl, lpb, repc, x0, cll, clb


def kernel(y_pred, labels, input_length, label_length):
    global _cached_nc
    y_pred = np.asarray(y_pred, np.float32)
    labels = np.asarray(labels, np.int32)
    input_length = np.asarray(input_length, np.int32)
    label_length = np.asarray(label_length, np.int32)
    lpl, lpb, repc, x0, cll, clb = _host_prep(
        y_pred, labels, input_length, label_length)
    if _cached_nc is None:
        _cached_nc = build_bass()
    in_maps = []
    for i in range(NCORES):
        sl = slice(i * BL, (i + 1) * BL)
        in_maps.append({"lpl": lpl[sl], "lpb": lpb[sl], "rep": repc[sl],
                        "x0": x0[sl], "cll": cll[sl], "clb": clb[sl]})
    res = run_bass_kernel_spmd(_cached_nc, in_maps, list(range(NCORES)))
    out = np.concatenate([res.results[i]["out"] for i in range(NCORES)], axis=0)
    return out.astype(np.float32)


# revision 10
# speedup vs baseline: 1.4300x; 1.0695x over previous
"""CTC batch loss on 8 TRN2 NeuronCores — v7: parity-split merged chains.

Like v6 (fwd alpha + bwd beta chains meeting at t*=255, QSP-LSE custom DVE
ops, fused inject+emission op), but the extended-state row is split by
parity: blank states (even s) never take the s-2 skip path, so they need
only an LSE2 + emission (2 instructions over 132 cols) while labels
(odd s) run the full LSE3 path (4 instructions over 131 cols). Total
per-step DVE elements drop from 4x261=1044 to 4x131+2x132=788.

Layout (state row, width 268):
  cols 0,1   pad NEG
  cols 2..65    fwd labels l_k  (k=0..63, s=2k+1)
  cols 66..68   pad
  cols 69..132  bwd labels (reversed): gl_k at col 132-k
  cols 133,134  pad (never written)
  cols 135..199 fwd blanks b_k  (k=0..64, s=2k)
  cols 200,201  pad
  cols 202..266 bwd blanks (reversed): gb_k at col 266-k
  col 267    pad

Recurrences (g = beta + lp for the bwd chain, all QSP-approximated):
  fwd: l_k' = lp_l + LSE3(l_k, b_k, l_{k-1}*rep_k);  b_k' = lp_b + LSE2(b_k, l_{k-1})
  bwd: gl_k' = lp_l + LSE3(gl_k, gb_{k+1}, gl_{k+1}*rep_{k+1});  gb_k' = lp_b + LSE2(gb_k, gl_k)
Both halves of each group share one instruction window; the reversed bwd
layout makes all relative offsets match the fwd ones.
"""
import sys

for _p in ("/opt/trn_rl_repo", "/opt/pypackages"):
    if _p not in sys.path:
        sys.path.insert(0, _p)

import numpy as np
import ml_dtypes

import concourse.bass as bass
import concourse.tile as tile
from concourse import mybir
from concourse.bass_utils import run_bass_kernel_spmd

B, T, C, L = 1024, 512, 128, 64
S = 2 * L + 1
NCORES = 8
BL = B // NCORES
EPS = 1e-7
NEG = -30000.0

TW = 264               # state row width: fwd s at col 2+s, bwd s at col 262-s
W = 261                # full-row window: out cols [2, 263)
NLAB = 130             # odd (label) sublattice elements: cols 3,5,...,261
NBLK = 131             # even (blank) sublattice elements: cols 2,4,...,262
LPW = 132              # per-step lp stream stride (both sublattices)
NSTEP = 256
CHUNKS = [4, 12, 16, 32, 32, 32, 32, 32, 32, 32]
assert sum(CHUNKS) == NSTEP
CINJ_OFF = 9999.0

F32 = mybir.dt.float32
BF16 = mybir.dt.bfloat16
ALU = mybir.AluOpType

SP_C0 = 0.8129
SP_C1 = -0.2261
INJ_BIG = -3.0e38

_MAX_WAITS = 1


def _patched_drain_and_barrier(self, tick_clock, wait_clock):
    from concourse.vector_clock import ScopedClock

    drain_inst = self.nc.sync.drain()
    wait_clock.add_sem_waits(
        drain_inst.ins, ScopedClock({None: tick_clock.global_clock})
    )
    si = drain_inst.ins.sync_info
    waits = list(si.on_wait) if si and si.on_wait else []
    if len(waits) > _MAX_WAITS:
        drain_inst.ins.sync_info = mybir.SyncInfo(
            on_wait=waits[:_MAX_WAITS], on_update=list(si.on_update or [])
        )
        for i in range(_MAX_WAITS, len(waits), _MAX_WAITS):
            extra = self.nc.sync.drain()
            extra.ins.sync_info = mybir.SyncInfo(
                on_wait=waits[i:i + _MAX_WAITS], on_update=[]
            )

    self.nc.all_engine_barrier()
    assert self.sems is not None
    popped = self.nc._tile_sem_poison_stack.pop()
    assert popped is self._sem_poison
    self.nc.clear_and_free_semaphores(list(self.sems.allocated().values()))
    self.nc.all_engine_barrier()


tile.TileContext._drain_and_barrier = _patched_drain_and_barrier


def _split_multiwait_bir(ant_bir) -> bytes:
    import json as _json

    bir = _json.loads(ant_bir)
    for f in bir.get("functions", []):
        for blk in f.get("blocks", []):
            out = []
            for ins in blk.get("instructions", []):
                si = ins.get("sync_info")
                waits = (si or {}).get("on_wait") or []
                if len(waits) > 1:
                    for j, w in enumerate(waits[:-1]):
                        out.append({
                            "debug": ins.get("debug", 0),
                            "engine": ins["engine"],
                            "ins": [],
                            "name": f"{ins['name']}_w{j}",
                            "opcode": "Drain",
                            "outs": [],
                            "sync_info": {"on_update": [], "on_wait": [w]},
                        })
                    si["on_wait"] = [waits[-1]]
                out.append(ins)
            blk["instructions"] = out
    return _json.dumps(bir).encode()


def _install_bir_splitter():
    import concourse.bass_utils as _bu
    import concourse.bass2jax as _b2j

    orig = _bu.compile_bir_kernel
    if getattr(orig, "_multiwait_patched", False):
        return

    def patched(ant_bir_str, compile_dir_path, neff_name="file.neff", **kw):
        return orig(_split_multiwait_bir(ant_bir_str), compile_dir_path,
                    neff_name=neff_name, **kw)

    patched._multiwait_patched = True
    _bu.compile_bir_kernel = patched
    if hasattr(_b2j, "compile_bir_kernel"):
        _b2j.compile_bir_kernel = patched


_install_bir_splitter()


def _lse_ref(in0, in1, s0, s1, imm2):
    a = np.asarray(in0, np.float32)
    b = np.asarray(in1, np.float32)
    m = np.maximum(a, b)
    t = m - np.minimum(a, b)
    return (m + np.maximum(s0 + s1 * t, 0.0) ** 2).astype(np.float32)


def _inj1_ref(in0, in1, s0, s1, imm2):
    a = np.asarray(in0, np.float32)
    lp = np.asarray(in1, np.float32)
    k = np.arange(a.shape[-1], dtype=np.float32)[None, :]
    u = k - (s0 if isinstance(s0, float) else np.asarray(s0, np.float32))
    inj = np.minimum(u * u, 1.0) * (s1 if isinstance(s1, float)
                                    else np.asarray(s1, np.float32))
    return (np.maximum(a, inj) + lp).astype(np.float32)


_OPS = None


def _make_ops():
    global _OPS
    if _OPS is not None:
        return _OPS
    from concourse import dve_ops as dops
    from concourse.dve_spec import (Spec, Src0, Src1, C0, C1, One, Idx,
                                    relu, sq, maxx, minn, lower)
    from concourse.dve_spec import _has_src1
    from concourse.dve_uop import DveOpSpec

    def register(name, body, ref):
        for existing in dops.OPS:
            if existing.name == name:
                return existing
        spec = Spec(body=body, reference=ref)
        row = dops._CUSTOM_DVE_ROW_BASE + len(dops.OPS)
        shas = {}
        for ver in ("v3", "v4"):
            uops = lower(spec, ver=ver)
            tmp = DveOpSpec(name=name, opcode=row, uops=uops,
                            rd1_en=_has_src1(spec))
            shas[ver] = tmp.sha(ver)
        op = dops.DveOp(name, spec, subdim=False, uops_sha=shas)
        dops.OPS.append(op)
        dops._SUB_OPCODE_FOR_NAME[name] = row
        dops.CUSTOM_DVE_SPECS[name] = spec
        return op

    m = maxx(Src0, Src1)
    n = minn(Src0, Src1)
    lse_body = m + sq(relu(C0 + C1 * (m - n)))
    lse_op = register("LSE_QSP_ANT", lse_body, _lse_ref)

    # single-column inject + emission add: out = max(Src0, w) + Src1 where
    # w = 0.0 exactly at Idx==C0 and C1=-3e38 elsewhere (C0=9999: no-op).
    u = Idx - C0
    inj1_body = maxx(Src0, minn(sq(u), One) * C1) + Src1
    inj1_op = register("INJ1_ANT", inj1_body, _inj1_ref)

    _OPS = (lse_op, inj1_op)
    return _OPS


_cached_nc = None


def build_bass():
    lse_op, inj1_op = _make_ops()
    nc = bass.Bass()
    lpl_d = nc.declare_dram_parameter("lpl", [BL, NSTEP * LPW], BF16, isOutput=False)
    lpb_d = nc.declare_dram_parameter("lpb", [BL, NSTEP * LPW], BF16, isOutput=False)
    rep_d = nc.declare_dram_parameter("rep", [BL, LPW], F32, isOutput=False)
    x0_d = nc.declare_dram_parameter("x0", [BL, TW], F32, isOutput=False)
    cll_d = nc.declare_dram_parameter("cll", [BL, NSTEP], F32, isOutput=False)
    out_d = nc.declare_dram_parameter("out", [BL, 1], F32, isOutput=True)

    with tile.TileContext(nc) as tc:
        with (
            tc.tile_pool(name="lpp", bufs=1) as lp_pool,
            tc.tile_pool(name="persist", bufs=1) as pp,
        ):
            x_a = pp.tile([BL, TW], F32, tag="x_a")
            x_b = pp.tile([BL, TW], F32, tag="x_b")
            lrx_a = pp.tile([BL, LPW], F32, tag="lrx_a")
            lrx_b = pp.tile([BL, LPW], F32, tag="lrx_b")
            l1t = pp.tile([BL, TW], F32, tag="l1t")
            l2t = pp.tile([BL, LPW], F32, tag="l2t")
            rept = pp.tile([BL, LPW], F32, tag="rept")
            cllt = pp.tile([BL, NSTEP], F32, tag="cllt")
            am = pp.tile([BL, 136], F32, tag="am")
            sc = pp.tile([BL, 176], F32, tag="sc")
            loss = pp.tile([BL, 1], F32, tag="loss")

            nc.vector.memset(x_b[:, :], NEG)
            nc.vector.memset(am[:, :], NEG)
            nc.vector.memset(sc[:, :], NEG)
            nc.sync.dma_start(out=x_a[:, :], in_=x0_d[:, :])
            nc.sync.dma_start(out=rept[:, :], in_=rep_d[:, :])
            nc.sync.dma_start(out=cllt[:, :], in_=cll_d[:, :])
            lpts = []
            lo = 0
            for ci, csz in enumerate(CHUNKS):
                lplt = lp_pool.tile([BL, csz * LPW], BF16, tag=f"lpl{ci}")
                nc.sync.dma_start(out=lplt[:, :],
                                  in_=lpl_d[:, lo * LPW:(lo + csz) * LPW])
                lpbt = lp_pool.tile([BL, csz * LPW], BF16, tag=f"lpb{ci}")
                nc.sync.dma_start(out=lpbt[:, :],
                                  in_=lpb_d[:, lo * LPW:(lo + csz) * LPW])
                lpts.append((lplt, lpbt, lo, csz))
                lo += csz

            xc, xn = x_a, x_b
            lrc, lrn = lrx_a, lrx_b
            for lplt, lpbt, lo, csz in lpts:
                for il in range(csz):
                    i = lo + il
                    # gated label skip path on GPSIMD (odd sublattice): runs
                    # in the shadow of the first two DVE ops below.
                    nc.gpsimd.tensor_add(lrc[:, 0:NLAB], xc[:, 1:261:2],
                                         rept[:, 0:NLAB])
                    # DVE: one full-row LSE2 serves both parities
                    nc.vector._custom_dve(lse_op, out=l1t[:, 2:2 + W],
                                          in0=xc[:, 2:2 + W],
                                          in1=xc[:, 1:1 + W],
                                          s0=SP_C0, s1=SP_C1)
                    # blanks are done: inject + emission on the even cols
                    nc.vector._custom_dve(inj1_op, out=xn[:, 2:263:2],
                                          in0=l1t[:, 2:263:2],
                                          in1=lpbt[:, il * LPW: il * LPW + NBLK],
                                          s0=cllt[:, i:i + 1], s1=INJ_BIG)
                    # labels: second LSE2 against the gated skip path
                    nc.vector._custom_dve(lse_op, out=l2t[:, 0:NLAB],
                                          in0=l1t[:, 3:263:2],
                                          in1=lrc[:, 0:NLAB],
                                          s0=SP_C0, s1=SP_C1)
                    nc.vector._custom_dve(inj1_op, out=xn[:, 3:263:2],
                                          in0=l2t[:, 0:NLAB],
                                          in1=lplt[:, il * LPW: il * LPW + NLAB],
                                          s0=cllt[:, i:i + 1], s1=INJ_BIG)
                    xc, xn = xn, xc
                    lrc, lrn = lrn, lrc

            # readout: alpha[s] at col 2+s, beta[s] at col 262-s
            nc.vector.tensor_add(am[:, 0:S], xc[:, 2:2 + S],
                                 xc[:, 262:133:-1])

            def tree(out_o, in_t, in_o, wlo):
                nc.vector._custom_dve(
                    lse_op, out=sc[:, out_o:out_o + wlo],
                    in0=in_t[:, in_o:in_o + wlo],
                    in1=in_t[:, in_o + wlo:in_o + 2 * wlo],
                    s0=SP_C0, s1=SP_C1)

            tree(0, am, 0, 65)      # 129 -> 65  (am[129]=NEG)
            tree(80, sc, 0, 33)     # 65 -> 33   (sc[65]=NEG)
            tree(120, sc, 80, 17)   # 33 -> 17   (sc[113]=NEG)
            tree(140, sc, 120, 9)   # 17 -> 9    (sc[137]=NEG)
            tree(152, sc, 140, 5)   # 9 -> 5     (sc[149]=NEG)
            tree(160, sc, 152, 3)   # 5 -> 3     (sc[157]=NEG)
            tree(168, sc, 160, 2)   # 3 -> 2     (sc[163]=NEG)
            tree(172, sc, 168, 1)   # 2 -> 1
            nc.vector.tensor_scalar_mul(loss[:, 0:1], sc[:, 172:173], -1.0)
            nc.sync.dma_start(out=out_d[:, :], in_=loss[:, 0:1])
    mybir.codegen_inst_isa_subclasses(nc)
    return nc


def _host_prep(y_pred, labels, input_length, label_length):
    blank = C - 1
    lab = labels.astype(np.int64)
    q_l = np.take_along_axis(y_pred, lab[:, None, :], axis=2)   # [B,T,64]
    lp_l = np.log(q_l.astype(np.float32) + EPS)                 # label lp
    lp_b = np.log(y_pred[:, :, blank].astype(np.float32) + EPS) # [B,T] blank lp
    frozen = np.arange(T)[None, :] >= input_length[:, None]
    lp_l[frozen, :] = 0.0
    lp_b[frozen] = 0.0

    # rep gate for label k vs k-1 (fwd: gates l_{k-1} -> l_k)
    rep = np.full((B, L), 0.0, np.float32)
    rep[:, 1:] = np.where(labels[:, 1:] != labels[:, :-1], 0.0, NEG)
    # rep[:, 0] = 0.0: the gated read hits the NEG pad anyway.

    lens = input_length.astype(np.int64)
    llen = label_length.astype(np.int64)                        # [B] in [32,64]

    # odd-sublattice lp stream: e=0..63 fwd labels (out col 3+2e, s=1+2e),
    # e=64,65 pads, e=66..129 bwd labels (s=259-2e -> k=129-e), lp[510-i]
    lpl = np.full((B, NSTEP, LPW), NEG, np.float32)
    lpl[:, :, 0:64] = lp_l[:, 0:NSTEP, :]
    lpl[:, 0:NSTEP - 1, 66:130] = lp_l[:, 510:255:-1, ::-1]
    lpl[:, NSTEP - 1, 66:130] = 0.0
    lpl = lpl.reshape(B, NSTEP * LPW).astype(ml_dtypes.bfloat16)

    # even-sublattice lp stream: e=0..64 fwd blanks (col 2+2e), e=65 pad
    # (col 132), e=66..130 bwd blanks (col 2+2e = 134..262)
    lpb = np.full((B, NSTEP, LPW), NEG, np.float32)
    lpb[:, :, 0:65] = lp_b[:, 0:NSTEP, None]
    lpb[:, 0:NSTEP - 1, 66:131] = lp_b[:, 510:255:-1, None]
    lpb[:, NSTEP - 1, 66:131] = 0.0
    lpb = lpb.reshape(B, NSTEP * LPW).astype(ml_dtypes.bfloat16)

    # rep stream on the odd sublattice: e=0..63 fwd rep_e; e=64..66 pads;
    # e=67..129 bwd label k=129-e gated by rep_{k+1} = rep_{130-e}
    repc = np.full((B, LPW), NEG, np.float32)
    repc[:, 0:64] = rep
    repc[:, 67:130] = rep[:, 1:64][:, ::-1]

    # initial state: fwd s at col 2+s, bwd s at col 262-s
    x0 = np.full((B, TW), NEG, np.float32)
    x0[:, 2] = 0.0                                              # alpha seed s=0
    bi = np.nonzero(lens == 512)[0]
    x0[bi, 262 - 2 * llen[bi]] = lp_b[bi, 511]                  # s_last (blank)
    x0[bi, 263 - 2 * llen[bi]] = lp_l[bi, 511, llen[bi] - 1]    # s_last-1
    # injection table (shared by both sublattices: element 130-llen)
    cl = np.full((B, NSTEP), CINJ_OFF, np.float32)
    ii = 511 - lens
    has = (ii >= 0) & (ii <= 255)
    bi = np.nonzero(has)[0]
    cl[bi, ii[bi]] = (130 - llen[bi]).astype(np.float32)

    return lpl, lpb, repc, x0, cl


def kernel(y_pred, labels, input_length, label_length):
    global _cached_nc
    y_pred = np.asarray(y_pred, np.float32)
    labels = np.asarray(labels, np.int32)
    input_length = np.asarray(input_length, np.int32)
    label_length = np.asarray(label_length, np.int32)
    lpl, lpb, repc, x0, cl = _host_prep(
        y_pred, labels, input_length, label_length)
    if _cached_nc is None:
        _cached_nc = build_bass()
    in_maps = []
    for i in range(NCORES):
        sl = slice(i * BL, (i + 1) * BL)
        in_maps.append({"lpl": lpl[sl], "lpb": lpb[sl], "rep": repc[sl],
                        "x0": x0[sl], "cll": cl[sl]})
    res = run_bass_kernel_spmd(_cached_nc, in_maps, list(range(NCORES)))
    out = np.concatenate([res.results[i]["out"] for i in range(NCORES)], axis=0)
    return out.astype(np.float32)
